# revision 49
# baseline (speedup 1.0000x reference)
"""Multi-head causal attention (B=4, S=2048, D=1024, H=16) on 8 trn2 cores.

Sharding: tensor-parallel over heads x data-parallel over batch.
core c -> (batch b = c//2, head-group hg = c%2 of 8 heads). Every core runs
an identical SPMD program on different data:
  - QKV projections for its 512 features (8 heads). K kept transposed
    [feat, seq] in SBUF; V kept [seq, feat] with a 64-wide ones block per
    head (the PV matmul then emits the softmax denominator for free); Q
    produced per 512-query superblock just in time. Weights live in SBUF
    for the whole kernel (one 1 MB DMA each); x arrives as one prepacked
    DMA per (projection, superblock). Head transfers are split across the
    two hardware DMA queues (ScalarE queue: k/q, which also gate the first
    exps; sync queue: v, ones, and the sb1 x prefetch) since each queue
    executes its transfers serially in emission order.
  - Causal attention per (head, superblock): S^T = K^T.T @ Q^T per 128-key
    block as a single N<=512 bf16 matmul, trimmed to the causally-visible
    query range for diagonal blocks; exp on ScalarE (scores are O(5), exp
    cannot overflow, so no max subtraction) batched over [128,1024] PSUM
    pairs to amortize the 352-cycle ACTIVATE overhead -- the diagonal
    four are ordered (d3,d0,d1,d2) so the first pair's trimmed regions
    are contiguous and share one ACTIVATE; in-place [128,128] triangular
    mask multiply on the partial chunk of diagonal blocks; PV accumulation
    in PSUM (d0 full-width opens the group, trimmed rest). Score and PV
    batches are software-pipelined (PV of batch b is emitted after the
    scores of batch b+1) so the TensorE never waits on the exp stream.
  - ctx^T written straight into persistent SBUF tiles (even heads by DVE
    output placement, odd heads via a small SBUF->SBUF partition-shift DMA)
    -- no DRAM round-trip. Output projection reads those tiles directly,
    and is scheduled late (sb3 + a post-stream remnant) to fill the TensorE
    while the exp-heavy attention tail runs on ScalarE.
Host sums the two partial outputs per batch (the "all-reduce after W_o"
done at gather time) and folds the Wo @ bv + bo constant.

All matmuls run in bf16 (measured 216 ns per N=512 matmul vs 227+ for
f32r, LDWEIGHTS hidden except across PE-geometry switches, no K=64 or
small-N penalties); PSUM accumulation stays fp32, so only input rounding
(~0.4%) is lost, well inside the 2e-2 gate. K=128 matmul density is kept
spread out (no clumping) because dense K=128 bursts trip the HAM 0.5-duty
throttle; K=64 score matmuls don't count toward it.
"""

import sys

import numpy as np

_BASS_PATH = "/opt/trn_rl_repo"
if _BASS_PATH not in sys.path:
    sys.path.insert(0, _BASS_PATH)

B, S, D, H, DK = 4, 2048, 1024, 16, 64
NCORES = 8
FH = 512  # features per core (8 heads)
HL = 8  # local heads
NSC = 4  # seq superblocks of 512
SQ = 512
NKB = 16  # key blocks of 128
NDM = 8  # d_model chunks of 128

_cache = {}


def _build():
    import concourse.bacc as bacc
    import concourse.mybir as mybir
    from concourse.tile import TileContext

    f32, bf16 = mybir.dt.float32, mybir.dt.bfloat16
    AF = mybir.ActivationFunctionType

    nc = bacc.Bacc("TRN2", target_bir_lowering=False, debug=False, num_devices=1)

    # x prepacked as [sc, p, dm*SQ]; w prepacked as [p, dm*FH]
    xq_d = nc.dram_tensor("xq", [NSC, 128, NDM * SQ], bf16, kind="ExternalInput").ap()
    xk_d = nc.dram_tensor("xk", [NSC, 128, NDM * SQ], bf16, kind="ExternalInput").ap()
    xv_d = nc.dram_tensor("xv", [NSC, 128, NDM * SQ], bf16, kind="ExternalInput").ap()
    wq_d = nc.dram_tensor("wq", [128, NDM * FH], bf16, kind="ExternalInput").ap()
    wk_d = nc.dram_tensor("wk", [128, NDM * FH], bf16, kind="ExternalInput").ap()
    wv_d = nc.dram_tensor("wv", [128, NDM * FH], bf16, kind="ExternalInput").ap()
    wo_d = nc.dram_tensor("wo", [128, 4 * D], bf16, kind="ExternalInput").ap()
    # triangular causal mask [128, 128]: tri[k, c] = 1 iff k <= c
    tri_d = nc.dram_tensor("tri", [128, 128], bf16, kind="ExternalInput").ap()
    # packed biases [128, 8]: cols 0-3 = bq chunks, cols 4-7 = bk chunks
    bias_d = nc.dram_tensor("bias", [128, 8], f32, kind="ExternalInput").ap()
    ones_d = nc.dram_tensor("ones", [128, SQ], bf16, kind="ExternalInput").ap()
    out_d = nc.dram_tensor("out", [S, D], bf16, kind="ExternalOutput").ap()

    with TileContext(nc) as tc:
        with (
            tc.tile_pool(name="res", bufs=1) as res,
            tc.tile_pool(name="st", bufs=1) as st,
            tc.tile_pool(name="psum", bufs=1, space="PSUM") as psp,
        ):
            # ---- persistent tiles + head DMAs (ordered for earliest start)
            # tiny consts first so they don't queue behind the 6 MB of x/w
            tri = res.tile([128, 128], bf16, name="tri", tag="tri")
            nc.sync.dma_start(tri[:], tri_d[:])
            bias_sb = res.tile([128, 8], f32, name="bias", tag="bias")
            nc.sync.dma_start(bias_sb[:], bias_d[:])
            bq_t = [bias_sb[:, i : i + 1] for i in range(4)]
            bk_t = [bias_sb[:, 4 + i : 5 + i] for i in range(4)]

            w_sb = {}
            xbox = {}
            HX = NDM * SQ // 2

            def emit_x_load(pname, x_d, sc, halves=False):
                xt = st.tile(
                    [128, NDM * SQ], bf16, name=f"x{pname}", tag=f"x{pname}", bufs=2
                )
                if halves:
                    nc.sync.dma_start(xt[:, 0:HX], x_d[sc][:, 0:HX])
                    nc.sync.dma_start(xt[:, HX:], x_d[sc][:, HX:])
                else:
                    nc.sync.dma_start(xt[:], x_d[sc])
                xbox[(pname, sc)] = xt

            # Two independent DMA queues (sync + scalar). Head plan:
            #   scalar queue: k and q w/x (gates the first scores AND the
            #     first exps, so borrowing ScalarE costs nothing)
            #   sync queue: v w/x, ones blocks, then sc=1's x prefetches
            # so sb1 never stalls behind the head transfers.
            for pname, w_d, x_d in (("k", wk_d, xk_d), ("q", wq_d, xq_d)):
                wt = res.tile([128, NDM * FH], bf16, name=f"w{pname}", tag=f"w{pname}")
                nc.scalar.dma_start(wt[:, 0:HX], w_d[:, 0:HX])
                w_sb[pname] = wt
                xt = st.tile(
                    [128, NDM * SQ], bf16, name=f"x{pname}", tag=f"x{pname}", bufs=2
                )
                nc.scalar.dma_start(xt[:, 0:HX], x_d[0][:, 0:HX])
                xbox[(pname, 0)] = xt
                nc.scalar.dma_start(wt[:, HX:], w_d[:, HX:])
                nc.scalar.dma_start(xt[:, HX:], x_d[0][:, HX:])
            wt = res.tile([128, NDM * FH], bf16, name="wv", tag="wv")
            nc.sync.dma_start(wt[:, 0:HX], wv_d[:, 0:HX])
            w_sb["v"] = wt
            xt = st.tile([128, NDM * SQ], bf16, name="xv", tag="xv", bufs=2)
            nc.sync.dma_start(xt[:, 0:HX], xv_d[0][:, 0:HX])
            xbox[("v", 0)] = xt
            nc.sync.dma_start(wt[:, HX:], wv_d[:, HX:])
            nc.sync.dma_start(xt[:, HX:], xv_d[0][:, HX:])
            # wo is not needed until o(0) in sb2; loaded with sb1's warm
            wo_sb = res.tile([128, 4 * D], bf16, name="wo", tag="wo")

            kt = [res.tile([128, S], bf16, name=f"kt{i}", tag=f"kt{i}") for i in range(4)]
            # per head: 64 V columns + 64 ones columns. The PV matmul then
            # yields the softmax denominator replicated on partitions 64-127
            # (no partition_broadcast needed for the normalize).
            vaug = [
                res.tile([128, HL * 128], bf16, name=f"va{k}", tag=f"va{k}")
                for k in range(NKB)
            ]
            # persistent ctx^T tiles: ctx_sb[sb][fc] is [128, 512]
            ctx_sb = [
                [
                    res.tile([128, SQ], bf16, name=f"cx{sb}{fc}", tag=f"cx{sb}{fc}")
                    for fc in range(4)
                ]
                for sb in range(NSC)
            ]
            # ones blocks land via DMA (no compute-engine queue time). The
            # head region is chip-HBM-bound, so order strictly by need time:
            # kb0-7 (sb0 + sb1-start PVs), then sb1's x prefetch, then the
            # rest of the ones blocks.
            ones_src = ones_d.rearrange("p (h o) -> p h o", o=64)

            def ones_dma(kb, eng):
                va3 = vaug[kb][:, :].rearrange("p (h e) -> p h e", e=128)
                eng.dma_start(va3[:, :, 64:128], ones_src)

            for kb in range(8):
                ones_dma(kb, nc.sync)
            emit_x_load("k", xk_d, 1)
            emit_x_load("v", xv_d, 1)
            emit_x_load("q", xq_d, 1)
            # late ones ride the scalar DMA queue (idle until the first exp)
            for kb in range(8, NKB):
                ones_dma(kb, nc.scalar)

            qsf_box = {}  # sc -> [4 q-slice tiles]

            def make_proj_thunk(pname, sc, gi):
                def group(pname=pname, sc=sc, gi=gi):
                    if pname == "q" and gi == 0:
                        qsf_box[sc] = [
                            st.tile(
                                [128, SQ], bf16, name=f"qs{i}", tag=f"qs{i}", bufs=2
                            )
                            for i in range(4)
                        ]
                    xt = xbox[(pname, sc)]
                    w = w_sb[pname]
                    if pname in ("q", "k"):
                        pp = psp.tile([128, SQ], f32, name="pp", tag="pp", bufs=2)
                        for dm in range(NDM):
                            c0 = dm * FH + gi * 128
                            nc.tensor.matmul(
                                pp[:],
                                w[:, c0 : c0 + 128],
                                xt[:, dm * SQ : (dm + 1) * SQ],
                                start=(dm == 0),
                                stop=(dm == NDM - 1),
                            )
                        if pname == "k":
                            nc.vector.tensor_scalar_add(
                                kt[gi][:, sc * SQ : (sc + 1) * SQ], pp[:], bk_t[gi]
                            )
                        else:
                            nc.vector.tensor_scalar_add(
                                qsf_box[sc][gi][:], pp[:], bq_t[gi]
                            )
                    else:  # v
                        kb = sc * 4 + gi
                        pp = psp.tile([128, FH], f32, name="pp", tag="pp", bufs=2)
                        for dm in range(NDM):
                            c0 = dm * SQ + gi * 128
                            nc.tensor.matmul(
                                pp[:],
                                xt[:, c0 : c0 + 128],
                                w[:, dm * FH : (dm + 1) * FH],
                                start=(dm == 0),
                                stop=(dm == NDM - 1),
                            )
                        va3 = vaug[kb][:, :].rearrange("p (h e) -> p h e", e=128)
                        pp3 = pp[:].rearrange("p (h e) -> p h e", e=64)
                        nc.vector.tensor_copy(va3[:, :, 0:64], pp3[:])

                return group

            def make_proj_thunks(sc, order=None):
                if order is None:
                    # round-robin k/v/q so the q bias-adds (which gate the
                    # next superblock's first scores) land early in the DVE
                    # queue, and vaug tiles spread across the superblock
                    order = [
                        (pn, g) for g in range(4) for pn in ("k", "v", "q")
                    ]
                thunks = []
                loads_done = set()
                for pname, gi in order:
                    if pname not in loads_done and sc > 1:
                        # x load for this (pname, sc) rides with its first group
                        loads_done.add(pname)
                        xd = {"k": xk_d, "v": xv_d, "q": xq_d}[pname]

                        def lg(pname=pname, xd=xd, sc=sc, gi=gi):
                            emit_x_load(pname, xd, sc)
                            make_proj_thunk(pname, sc, gi)()

                        thunks.append(lg)
                    else:
                        thunks.append(make_proj_thunk(pname, sc, gi))
                return thunks

            def make_attn_pairs(h, sb):
                """(score_thunk, pv_thunk) pairs for one (head, superblock).

                kb order: diagonal blocks first [d0..d3] (d0 full query width
                and start=True; d1..d3 trimmed to the causally visible query
                range), then off-diagonal blocks.
                """
                ti, po = h // 2, (h % 2) * 64
                nkb = 4 * (sb + 1)
                # diagonal order (d3, d0, d1, d2): the first pair's trimmed
                # exp regions [384:512] and [512:1024] merge into ONE
                # contiguous ACTIVATE; PV still consumes d0 first (start=True
                # needs d0's full query width)
                b = 4 * sb
                kbs = [b + 3, b + 0, b + 1, b + 2] + list(range(4 * sb))
                state = {}

                def score(b0):
                    esl = []
                    for pi in range(2):  # two kb-pairs per batch
                        sp = psp.tile(
                            [128, 2 * SQ], f32, name="sp", tag="sp", bufs=2
                        )
                        es = st.tile(
                            [128, 2 * SQ], bf16, name="es", tag="es", bufs=8
                        )
                        q0s = []
                        for j in range(2):
                            kb = kbs[b0 + 2 * pi + j]
                            dj = kb - sb * 4  # diagonal index (0..3) or neg
                            q0 = dj * 128 if 0 <= dj else 0
                            nc.tensor.matmul(
                                sp[:, j * SQ + q0 : (j + 1) * SQ],
                                kt[ti][po : po + 64, kb * 128 : (kb + 1) * 128],
                                qsf_box[sb][ti][po : po + 64, q0:SQ],
                                start=True,
                                stop=True,
                            )
                            q0s.append((kb, dj, q0))
                        if all(dj < 0 for _, dj, _ in q0s):
                            # off-diagonal pair: one wide exp (352-cycle
                            # ACTIVATE overhead amortized over 1024 cols)
                            nc.scalar.activation(es[:], sp[:], AF.Exp)
                        elif q0s[0][1] == 3 and q0s[1][1] == 0:
                            # (d3, d0) pair: valid regions [384:512]+[512:1024]
                            # are contiguous -> one exp
                            nc.scalar.activation(
                                es[:, 384:1024], sp[:, 384:1024], AF.Exp
                            )
                        else:
                            for j, (kb, dj, q0) in enumerate(q0s):
                                nc.scalar.activation(
                                    es[:, j * SQ + q0 : (j + 1) * SQ],
                                    sp[:, j * SQ + q0 : (j + 1) * SQ],
                                    AF.Exp,
                                )
                        for j, (kb, dj, q0) in enumerate(q0s):
                            if 0 <= dj:
                                c = j * SQ + q0
                                nc.vector.tensor_mul(
                                    es[:, c : c + 128], es[:, c : c + 128], tri[:]
                                )
                            esl.append((kb, q0, es, j))
                    state[b0] = esl

                def pv(b0):
                    esl = state.pop(b0)
                    if b0 == 0:
                        state["cp"] = psp.tile(
                            [128, SQ], f32, name="cp", tag="cp", bufs=2
                        )
                        state["emitted"] = 0
                        # d0 (full query width) must open the accumulation
                        esl = [esl[1], esl[0], esl[2], esl[3]]
                    cp = state["cp"]
                    for kb, q0, es, j in esl:
                        nc.tensor.matmul(
                            cp[:, q0:SQ],
                            vaug[kb][:, h * 128 : (h + 1) * 128],
                            es[:, j * SQ + q0 : (j + 1) * SQ],
                            start=(state["emitted"] == 0),
                            stop=(state["emitted"] == nkb - 1),
                        )
                        state["emitted"] += 1
                    if b0 + 4 >= nkb:
                        # normalize and write ctx^T slice into SBUF ctx tiles
                        fc, prow = h // 2, (h % 2) * 64
                        d1 = st.tile([1, SQ], f32, name="d1", tag="d1", bufs=2)
                        nc.vector.tensor_copy(d1[:], cp[64:65, :])
                        rb = st.tile([64, SQ], f32, name="rb", tag="rb", bufs=2)
                        nc.gpsimd.partition_broadcast(rb[:], d1[:])
                        rc = st.tile([64, SQ], f32, name="rc", tag="rc", bufs=2)
                        nc.vector.reciprocal_approx_fast(rc[:], rb[:])
                        if prow == 0:
                            nc.vector.tensor_mul(
                                ctx_sb[sb][fc][0:64, :], cp[0:64, :], rc[:]
                            )
                        else:
                            nrm = st.tile(
                                [64, SQ], bf16, name="nrm", tag="nrm", bufs=2
                            )
                            nc.vector.tensor_mul(nrm[:], cp[0:64, :], rc[:])
                            # partition shift 0-63 -> 64-127 via SBUF->SBUF DMA
                            nc.sync.dma_start(ctx_sb[sb][fc][64:128, :], nrm[:])

                return [
                    (
                        (lambda b0=b0: score(b0)),
                        (lambda b0=b0: pv(b0)),
                    )
                    for b0 in range(0, nkb, 4)
                ]

            def make_o_thunks(sb, alt_cast=False):
                # alt_cast: alternate the PSUM->bf16 cast between DVE and
                # ScalarE -- in the kernel tail ScalarE is idle and the pp
                # double-buffer rotation is gated on these casts
                thunks = []
                for qb in range(4):
                    for n2 in range(2):

                        def group(qb=qb, n2=n2, sb=sb):
                            pp = psp.tile([128, SQ], f32, name="pp", tag="pp", bufs=2)
                            for fc in range(4):
                                nc.tensor.matmul(
                                    pp[:],
                                    ctx_sb[sb][fc][:, qb * 128 : (qb + 1) * 128],
                                    wo_sb[:, fc * D + n2 * SQ : fc * D + (n2 + 1) * SQ],
                                    start=(fc == 0),
                                    stop=(fc == 3),
                                )
                            ob = st.tile([128, SQ], bf16, name="ob", tag="ob", bufs=2)
                            if alt_cast and (qb * 2 + n2) % 2 == 1:
                                nc.scalar.copy(ob[:], pp[:])
                            else:
                                nc.vector.tensor_copy(ob[:], pp[:])
                            nc.sync.dma_start(
                                out_d[
                                    sb * SQ + qb * 128 : sb * SQ + (qb + 1) * 128,
                                    n2 * SQ : (n2 + 1) * SQ,
                                ],
                                ob[:],
                            )

                        thunks.append(group)
                return thunks

            def clump2(ts):
                # fuse warm thunks in pairs: fewer insertions into the
                # attention stream = fewer accumulation-group boundaries
                # (each boundary exposes an otherwise-hidden LDWEIGHTS)
                out = []
                for i in range(0, len(ts), 2):
                    chunk = ts[i : i + 2]
                    out.append(lambda chunk=chunk: [t() for t in chunk])
                return out

            def pipeline_stream(pairs_by_head, late_proj=None):
                # flatten to S_0, S_1, P_0, S_2, P_1, ... (lag-1 software
                # pipeline). late_proj[g] (this superblock's k/q projection
                # groups 1-3, deferred from the previous superblock) is
                # inserted one full head ahead of the first head that needs
                # it, so its DVE bias-add lands well before those scores.
                stream = []
                prev_p = None
                for hi, hpairs in enumerate(pairs_by_head):
                    if late_proj and hi in (1, 3, 5):
                        stream += late_proj[(hi + 1) // 2]
                    for s_t, p_t in hpairs:
                        stream.append(s_t)
                        if prev_p is not None:
                            stream.append(prev_p)
                        prev_p = p_t
                stream.append(prev_p)
                return stream

            def emit_stream(stream, warm):
                nb, nw = len(stream), len(warm)
                wi = 0
                for bi, bt in enumerate(stream):
                    bt()
                    while wi < nw and (wi + 1) * nb <= (bi + 1) * nw:
                        warm[wi]()
                        wi += 1
                while wi < nw:
                    warm[wi]()
                    wi += 1

            # ---- emission schedule ----
            # sb0: minimal prefix of proj(0) [k0 v0..v3 q0], then attention
            # heads interleaved explicitly with the remaining k/q groups
            # (head pair ti becomes eligible right after k_ti/q_ti), with
            # proj(1) woven across the whole superblock.
            p0 = {
                (pn, g): make_proj_thunk(pn, 0, g)
                for pn in ("k", "v", "q")
                for g in range(4)
            }
            for t in (p0[("k", 0)], p0[("q", 0)]):
                t()
            sp0 = [make_attn_pairs(h, 0)[0] for h in range(HL)]
            s0 = [p[0] for p in sp0]
            v0 = [p[1] for p in sp0]
            # scores for heads 0-1 run while the V projection's x/w still
            # stream in; k1/q1 sit AHEAD of the first PV in the tensor queue
            # so a V wait can never block the next head pair's projections
            stream0 = [
                s0[0], s0[1],
                p0[("v", 0)], p0[("v", 1)], p0[("v", 2)], p0[("v", 3)],
                p0[("k", 1)], p0[("q", 1)],
                s0[2], v0[0], s0[3], v0[1],
                p0[("k", 2)], p0[("q", 2)],
                s0[4], v0[2], s0[5], v0[3],
                p0[("k", 3)], p0[("q", 3)],
                s0[6], v0[4], s0[7], v0[5], v0[6], v0[7],
            ]
            # early proj set E(sc): k/q group 0 + all of v (what the next
            # superblock's first head pair and PVs need). Groups 1-3 of k/q
            # are deferred into the consuming superblock's own stream.
            EARLY = [("k", 0), ("v", 0), ("q", 0), ("v", 1), ("v", 2), ("v", 3)]
            emit_stream(stream0, make_proj_thunks(1, order=EARLY))

            def load_wo():
                nc.sync.dma_start(wo_sb[:], wo_d[:])

            o1 = make_o_thunks(1)
            o2 = make_o_thunks(2, alt_cast=True)
            for sb in (1, 2, 3):
                # sb3 ends on even head 6 so the final normalize writes ctx
                # directly (no partition-shift DMA on the critical tail)
                horder = [0, 1, 2, 3, 4, 5, 7, 6] if sb == 3 else range(HL)
                pairs_by_head = [make_attn_pairs(h, sb) for h in horder]
                late = {
                    g: [make_proj_thunk("k", sb, g), make_proj_thunk("q", sb, g)]
                    for g in (1, 2, 3)
                }
                warm = {
                    1: lambda: [load_wo] + make_proj_thunks(2, order=EARLY),
                    2: lambda: make_proj_thunks(3, order=EARLY) + make_o_thunks(0),
                    3: lambda: o1[:6] + o2[:4],
                }[sb]()
                emit_stream(pipeline_stream(pairs_by_head, late), warm)
            # leftover o(1)/o(2) groups fill the last normalize's latency
            for t in o1[6:] + o2[4:]:
                t()
            for t in make_o_thunks(3, alt_cast=True):
                t()

    nc.compile()
    return nc


def kernel(
    q,
    k,
    v,
    mask=None,
    Wq=None,
    bq=None,
    Wk=None,
    bk=None,
    Wv=None,
    bv=None,
    Wo=None,
    bo=None,
    **_unused,
):
    import ml_dtypes
    from concourse.bass_utils import run_bass_kernel_spmd

    if "nc" not in _cache:
        _cache["nc"] = _build()
    nc = _cache["nc"]

    bf = ml_dtypes.bfloat16
    q = np.asarray(q, np.float32)
    k = np.asarray(k, np.float32)
    v = np.asarray(v, np.float32)
    Wq = np.asarray(Wq, np.float32)
    Wk = np.asarray(Wk, np.float32)
    Wv = np.asarray(Wv, np.float32)
    Wo = np.asarray(Wo, np.float32)
    bq = np.zeros(D, np.float32) if bq is None else np.asarray(bq, np.float32)
    bk = np.zeros(D, np.float32) if bk is None else np.asarray(bk, np.float32)
    bv = np.zeros(D, np.float32) if bv is None else np.asarray(bv, np.float32)
    bo = np.zeros(D, np.float32) if bo is None else np.asarray(bo, np.float32)

    # triangular causal mask [128, 128]: tri[kk, c] = 1 iff kk <= c
    kk = np.arange(128)[:, None]
    cc = np.arange(128)[None, :]
    tri = (kk <= cc).astype(bf)

    def pack_x(x):  # [S, D] -> [NSC, 128, NDM*SQ]  (sc, p, dm*sq)
        xT = np.ascontiguousarray(x.T)  # [D, S]
        return np.ascontiguousarray(
            xT.reshape(NDM, 128, NSC, SQ).transpose(2, 1, 0, 3).reshape(
                NSC, 128, NDM * SQ
            )
        ).astype(bf)

    def pack_w(wT):  # [D, FH] -> [128, NDM*FH]  (p, dm*fh)
        return np.ascontiguousarray(
            wT.reshape(NDM, 128, FH).transpose(1, 0, 2).reshape(128, NDM * FH)
        ).astype(bf)

    xT = {}
    for b in range(B):
        xT[("q", b)] = pack_x(q[b])
        xT[("k", b)] = pack_x(k[b])
        xT[("v", b)] = pack_x(v[b])
    wqs, wks, wvs, wos, bqs = {}, {}, {}, {}, {}
    for hg in range(2):
        sl = slice(hg * FH, (hg + 1) * FH)
        wqs[hg] = pack_w(Wq[sl, :].T * np.float32(0.125))
        wks[hg] = pack_w(Wk[sl, :].T)
        wvs[hg] = pack_w(Wv[sl, :].T)
        # wo: [FH, D] -> [128, 4*D]  (p, fc*d)
        woT = Wo[:, sl].T
        wos[hg] = np.ascontiguousarray(
            woT.reshape(4, 128, D).transpose(1, 0, 2).reshape(128, 4 * D)
        ).astype(bf)
        # packed biases [128, 8]: cols 0-3 = scaled bq chunks, 4-7 = bk
        bb = np.empty((128, 8), np.float32)
        bb[:, 0:4] = (bq[sl] * np.float32(0.125)).reshape(4, 128).T
        bb[:, 4:8] = bk[sl].reshape(4, 128).T
        bqs[hg] = bb
    ones_arr = np.ones((128, SQ), bf)

    in_maps = []
    for c in range(NCORES):
        b, hg = c // 2, c % 2
        in_maps.append(
            {
                "xq": xT[("q", b)],
                "xk": xT[("k", b)],
                "xv": xT[("v", b)],
                "wq": wqs[hg],
                "wk": wks[hg],
                "wv": wvs[hg],
                "wo": wos[hg],
                "tri": tri,
                "bias": bqs[hg],
                "ones": ones_arr,
            }
        )

    res = run_bass_kernel_spmd(nc, in_maps, list(range(NCORES)))
    out = np.empty((B, S, D), np.float32)
    for b in range(B):
        out[b] = np.asarray(res.results[2 * b]["out"], np.float32) + np.asarray(
            res.results[2 * b + 1]["out"], np.float32
        )
    const = Wo @ bv + bo  # bv/bo contribution (folds exactly through softmax)
    if np.any(const):
        out += const[None, None, :]
    return out


# revision 52
# speedup vs baseline: 1.0358x; 1.0358x over previous
"""Multi-head causal attention (B=4, S=2048, D=1024, H=16) on 8 trn2 cores.

Sharding: tensor-parallel over heads x data-parallel over batch.
core c -> (batch b = c//2, head-group hg = c%2 of 8 heads). Every core runs
an identical SPMD program on different data:
  - QKV projections for its 512 features (8 heads). K kept transposed
    [feat, seq] in SBUF; V kept [seq, feat] with a 64-wide ones block per
    head (the PV matmul then emits the softmax denominator for free); Q
    produced per 512-query superblock just in time. Weights live in SBUF
    for the whole kernel (one 1 MB DMA each); x arrives as one prepacked
    DMA per (projection, superblock). Head transfers are split across the
    two hardware DMA queues (ScalarE queue: k/q, which also gate the first
    exps; sync queue: v, ones, and the sb1 x prefetch) since each queue
    executes its transfers serially in emission order.
  - Causal attention per (head, superblock): S^T = K^T.T @ Q^T per 128-key
    block as a single N<=512 bf16 matmul, trimmed to the causally-visible
    query range for diagonal blocks; exp on ScalarE (scores are O(5), exp
    cannot overflow, so no max subtraction) batched over [128,1024] PSUM
    pairs to amortize the 352-cycle ACTIVATE overhead -- the diagonal
    four are ordered (d3,d0,d1,d2) so the first pair's trimmed regions
    are contiguous and share one ACTIVATE; in-place [128,128] triangular
    mask multiply on the partial chunk of diagonal blocks; PV accumulation
    in PSUM (d0 full-width opens the group, trimmed rest). Score and PV
    batches are software-pipelined (PV of batch b is emitted after the
    scores of batch b+1) so the TensorE never waits on the exp stream.
  - ctx^T written straight into persistent SBUF tiles (even heads by DVE
    output placement, odd heads via a small SBUF->SBUF partition-shift DMA)
    -- no DRAM round-trip. Output projection reads those tiles directly,
    and is scheduled late (sb3 + a post-stream remnant) to fill the TensorE
    while the exp-heavy attention tail runs on ScalarE.
Host sums the two partial outputs per batch (the "all-reduce after W_o"
done at gather time) and folds the Wo @ bv + bo constant.

All matmuls run in bf16 (measured 216 ns per N=512 matmul vs 227+ for
f32r, LDWEIGHTS hidden except across PE-geometry switches, no K=64 or
small-N penalties); PSUM accumulation stays fp32, so only input rounding
(~0.4%) is lost, well inside the 2e-2 gate. K=128 matmul density is kept
spread out (no clumping) because dense K=128 bursts trip the HAM 0.5-duty
throttle; K=64 score matmuls don't count toward it.
"""

import sys

import numpy as np

_BASS_PATH = "/opt/trn_rl_repo"
if _BASS_PATH not in sys.path:
    sys.path.insert(0, _BASS_PATH)

B, S, D, H, DK = 4, 2048, 1024, 16, 64
NCORES = 8
FH = 512  # features per core (8 heads)
HL = 8  # local heads
NSC = 4  # seq superblocks of 512
SQ = 512
NKB = 16  # key blocks of 128
NDM = 8  # d_model chunks of 128

_cache = {}


def _build():
    import concourse.bacc as bacc
    import concourse.mybir as mybir
    from concourse.tile import TileContext

    f32, bf16 = mybir.dt.float32, mybir.dt.bfloat16
    AF = mybir.ActivationFunctionType

    nc = bacc.Bacc("TRN2", target_bir_lowering=False, debug=False, num_devices=1)

    # x prepacked as [sc, p, dm*SQ]; w prepacked as [p, dm*FH]
    xq_d = nc.dram_tensor("xq", [NSC, 128, NDM * SQ], bf16, kind="ExternalInput").ap()
    xk_d = nc.dram_tensor("xk", [NSC, 128, NDM * SQ], bf16, kind="ExternalInput").ap()
    xv_d = nc.dram_tensor("xv", [NSC, 128, NDM * SQ], bf16, kind="ExternalInput").ap()
    wq_d = nc.dram_tensor("wq", [128, NDM * FH], bf16, kind="ExternalInput").ap()
    wk_d = nc.dram_tensor("wk", [128, NDM * FH], bf16, kind="ExternalInput").ap()
    wv_d = nc.dram_tensor("wv", [128, NDM * FH], bf16, kind="ExternalInput").ap()
    wo_d = nc.dram_tensor("wo", [128, 4 * D], bf16, kind="ExternalInput").ap()
    # triangular causal mask [128, 128]: tri[k, c] = 1 iff k <= c
    tri_d = nc.dram_tensor("tri", [128, 128], bf16, kind="ExternalInput").ap()
    # packed biases [128, 8]: cols 0-3 = bq chunks, cols 4-7 = bk chunks
    bias_d = nc.dram_tensor("bias", [128, 8], f32, kind="ExternalInput").ap()
    ones_d = nc.dram_tensor("ones", [128, SQ], bf16, kind="ExternalInput").ap()
    out_d = nc.dram_tensor("out", [S, D], bf16, kind="ExternalOutput").ap()

    with TileContext(nc) as tc:
        with (
            tc.tile_pool(name="res", bufs=1) as res,
            tc.tile_pool(name="st", bufs=1) as st,
            tc.tile_pool(name="psum", bufs=1, space="PSUM") as psp,
        ):
            # ---- persistent tiles + head DMAs (ordered for earliest start)
            # tiny consts first so they don't queue behind the 6 MB of x/w
            tri = res.tile([128, 128], bf16, name="tri", tag="tri")
            nc.sync.dma_start(tri[:], tri_d[:])
            bias_sb = res.tile([128, 8], f32, name="bias", tag="bias")
            nc.sync.dma_start(bias_sb[:], bias_d[:])
            bq_t = [bias_sb[:, i : i + 1] for i in range(4)]
            bk_t = [bias_sb[:, 4 + i : 5 + i] for i in range(4)]

            w_sb = {}
            xbox = {}
            HX = NDM * SQ // 2

            def emit_x_load(pname, x_d, sc, halves=False):
                xt = st.tile(
                    [128, NDM * SQ], bf16, name=f"x{pname}", tag=f"x{pname}", bufs=2
                )
                if halves:
                    nc.sync.dma_start(xt[:, 0:HX], x_d[sc][:, 0:HX])
                    nc.sync.dma_start(xt[:, HX:], x_d[sc][:, HX:])
                else:
                    nc.sync.dma_start(xt[:], x_d[sc])
                xbox[(pname, sc)] = xt

            # Two independent DMA queues (sync + scalar). Head plan:
            #   scalar queue: k and q w/x (gates the first scores AND the
            #     first exps, so borrowing ScalarE costs nothing)
            #   sync queue: v w/x, ones blocks, then sc=1's x prefetches
            # so sb1 never stalls behind the head transfers.
            for pname, w_d, x_d in (("k", wk_d, xk_d), ("q", wq_d, xq_d)):
                wt = res.tile([128, NDM * FH], bf16, name=f"w{pname}", tag=f"w{pname}")
                nc.scalar.dma_start(wt[:, 0:HX], w_d[:, 0:HX])
                w_sb[pname] = wt
                xt = st.tile(
                    [128, NDM * SQ], bf16, name=f"x{pname}", tag=f"x{pname}", bufs=2
                )
                nc.scalar.dma_start(xt[:, 0:HX], x_d[0][:, 0:HX])
                xbox[(pname, 0)] = xt
                nc.scalar.dma_start(wt[:, HX:], w_d[:, HX:])
                nc.scalar.dma_start(xt[:, HX:], x_d[0][:, HX:])
            wt = res.tile([128, NDM * FH], bf16, name="wv", tag="wv")
            nc.sync.dma_start(wt[:, 0:HX], wv_d[:, 0:HX])
            w_sb["v"] = wt
            xt = st.tile([128, NDM * SQ], bf16, name="xv", tag="xv", bufs=2)
            nc.sync.dma_start(xt[:, 0:HX], xv_d[0][:, 0:HX])
            xbox[("v", 0)] = xt
            nc.sync.dma_start(wt[:, HX:], wv_d[:, HX:])
            nc.sync.dma_start(xt[:, HX:], xv_d[0][:, HX:])
            # wo is not needed until o(0) in sb2; loaded with sb1's warm
            wo_sb = res.tile([128, 4 * D], bf16, name="wo", tag="wo")

            kt = [res.tile([128, S], bf16, name=f"kt{i}", tag=f"kt{i}") for i in range(4)]
            # per head: 64 V columns + 64 ones columns. The PV matmul then
            # yields the softmax denominator replicated on partitions 64-127
            # (no partition_broadcast needed for the normalize).
            vaug = [
                res.tile([128, HL * 128], bf16, name=f"va{k}", tag=f"va{k}")
                for k in range(NKB)
            ]
            # persistent ctx^T tiles: ctx_sb[sb][fc] is [128, 512]
            ctx_sb = [
                [
                    res.tile([128, SQ], bf16, name=f"cx{sb}{fc}", tag=f"cx{sb}{fc}")
                    for fc in range(4)
                ]
                for sb in range(NSC)
            ]
            # ones blocks land via DMA (no compute-engine queue time). The
            # head region is chip-HBM-bound, so order strictly by need time:
            # kb0-7 (sb0 + sb1-start PVs), then sb1's x prefetch, then the
            # rest of the ones blocks.
            ones_src = ones_d.rearrange("p (h o) -> p h o", o=64)

            def ones_dma(kb):
                va3 = vaug[kb][:, :].rearrange("p (h e) -> p h e", e=128)
                nc.sync.dma_start(va3[:, :, 64:128], ones_src)

            for kb in range(8):
                ones_dma(kb)
            emit_x_load("k", xk_d, 1)
            emit_x_load("v", xv_d, 1)
            emit_x_load("q", xq_d, 1)
            for kb in range(8, NKB):
                ones_dma(kb)

            qsf_box = {}  # sc -> [4 q-slice tiles]

            def make_proj_thunk(pname, sc, gi):
                def group(pname=pname, sc=sc, gi=gi):
                    if pname == "q" and gi == 0:
                        qsf_box[sc] = [
                            st.tile(
                                [128, SQ], bf16, name=f"qs{i}", tag=f"qs{i}", bufs=2
                            )
                            for i in range(4)
                        ]
                    xt = xbox[(pname, sc)]
                    w = w_sb[pname]
                    if pname in ("q", "k"):
                        pp = psp.tile([128, SQ], f32, name="pp", tag="pp", bufs=2)
                        for dm in range(NDM):
                            c0 = dm * FH + gi * 128
                            nc.tensor.matmul(
                                pp[:],
                                w[:, c0 : c0 + 128],
                                xt[:, dm * SQ : (dm + 1) * SQ],
                                start=(dm == 0),
                                stop=(dm == NDM - 1),
                            )
                        if pname == "k":
                            nc.vector.tensor_scalar_add(
                                kt[gi][:, sc * SQ : (sc + 1) * SQ], pp[:], bk_t[gi]
                            )
                        else:
                            nc.vector.tensor_scalar_add(
                                qsf_box[sc][gi][:], pp[:], bq_t[gi]
                            )
                    else:  # v
                        kb = sc * 4 + gi
                        pp = psp.tile([128, FH], f32, name="pp", tag="pp", bufs=2)
                        for dm in range(NDM):
                            c0 = dm * SQ + gi * 128
                            nc.tensor.matmul(
                                pp[:],
                                xt[:, c0 : c0 + 128],
                                w[:, dm * FH : (dm + 1) * FH],
                                start=(dm == 0),
                                stop=(dm == NDM - 1),
                            )
                        va3 = vaug[kb][:, :].rearrange("p (h e) -> p h e", e=128)
                        pp3 = pp[:].rearrange("p (h e) -> p h e", e=64)
                        nc.vector.tensor_copy(va3[:, :, 0:64], pp3[:])

                return group

            def make_proj_thunks(sc, order=None):
                if order is None:
                    # round-robin k/v/q so the q bias-adds (which gate the
                    # next superblock's first scores) land early in the DVE
                    # queue, and vaug tiles spread across the superblock
                    order = [
                        (pn, g) for g in range(4) for pn in ("k", "v", "q")
                    ]
                thunks = []
                loads_done = set()
                for pname, gi in order:
                    if pname not in loads_done and sc > 1:
                        # x load for this (pname, sc) rides with its first group
                        loads_done.add(pname)
                        xd = {"k": xk_d, "v": xv_d, "q": xq_d}[pname]

                        def lg(pname=pname, xd=xd, sc=sc, gi=gi):
                            emit_x_load(pname, xd, sc)
                            make_proj_thunk(pname, sc, gi)()

                        thunks.append(lg)
                    else:
                        thunks.append(make_proj_thunk(pname, sc, gi))
                return thunks

            def make_attn_pairs(h, sb):
                """(score_thunk, pv_thunk) pairs for one (head, superblock).

                kb order: diagonal blocks first [d0..d3] (d0 full query width
                and start=True; d1..d3 trimmed to the causally visible query
                range), then off-diagonal blocks.
                """
                ti, po = h // 2, (h % 2) * 64
                nkb = 4 * (sb + 1)
                # diagonal order (d3, d0, d1, d2): the first pair's trimmed
                # exp regions [384:512] and [512:1024] merge into ONE
                # contiguous ACTIVATE; PV still consumes d0 first (start=True
                # needs d0's full query width)
                b = 4 * sb
                kbs = [b + 3, b + 0, b + 1, b + 2] + list(range(4 * sb))
                state = {}

                def score(b0):
                    esl = []
                    for pi in range(2):  # two kb-pairs per batch
                        sp = psp.tile(
                            [128, 2 * SQ], f32, name="sp", tag="sp", bufs=2
                        )
                        es = st.tile(
                            [128, 2 * SQ], bf16, name="es", tag="es", bufs=8
                        )
                        q0s = []
                        for j in range(2):
                            kb = kbs[b0 + 2 * pi + j]
                            dj = kb - sb * 4  # diagonal index (0..3) or neg
                            q0 = dj * 128 if 0 <= dj else 0
                            nc.tensor.matmul(
                                sp[:, j * SQ + q0 : (j + 1) * SQ],
                                kt[ti][po : po + 64, kb * 128 : (kb + 1) * 128],
                                qsf_box[sb][ti][po : po + 64, q0:SQ],
                                start=True,
                                stop=True,
                            )
                            q0s.append((kb, dj, q0))
                        if all(dj < 0 for _, dj, _ in q0s):
                            # off-diagonal pair: one wide exp (352-cycle
                            # ACTIVATE overhead amortized over 1024 cols)
                            nc.scalar.activation(es[:], sp[:], AF.Exp)
                        elif q0s[0][1] == 3 and q0s[1][1] == 0:
                            # (d3, d0) pair: valid regions [384:512]+[512:1024]
                            # are contiguous -> one exp
                            nc.scalar.activation(
                                es[:, 384:1024], sp[:, 384:1024], AF.Exp
                            )
                        else:
                            for j, (kb, dj, q0) in enumerate(q0s):
                                nc.scalar.activation(
                                    es[:, j * SQ + q0 : (j + 1) * SQ],
                                    sp[:, j * SQ + q0 : (j + 1) * SQ],
                                    AF.Exp,
                                )
                        for j, (kb, dj, q0) in enumerate(q0s):
                            if 0 <= dj:
                                c = j * SQ + q0
                                nc.vector.tensor_mul(
                                    es[:, c : c + 128], es[:, c : c + 128], tri[:]
                                )
                            esl.append((kb, q0, es, j))
                    state[b0] = esl

                def pv(b0):
                    esl = state.pop(b0)
                    if b0 == 0:
                        state["cp"] = psp.tile(
                            [128, SQ], f32, name="cp", tag="cp", bufs=2
                        )
                        state["emitted"] = 0
                        # d0 (full query width) must open the accumulation
                        esl = [esl[1], esl[0], esl[2], esl[3]]
                    cp = state["cp"]
                    for kb, q0, es, j in esl:
                        nc.tensor.matmul(
                            cp[:, q0:SQ],
                            vaug[kb][:, h * 128 : (h + 1) * 128],
                            es[:, j * SQ + q0 : (j + 1) * SQ],
                            start=(state["emitted"] == 0),
                            stop=(state["emitted"] == nkb - 1),
                        )
                        state["emitted"] += 1
                    if b0 + 4 >= nkb:
                        # normalize and write ctx^T slice into SBUF ctx tiles
                        fc, prow = h // 2, (h % 2) * 64
                        d1 = st.tile([1, SQ], f32, name="d1", tag="d1", bufs=2)
                        nc.vector.tensor_copy(d1[:], cp[64:65, :])
                        rb = st.tile([64, SQ], f32, name="rb", tag="rb", bufs=2)
                        nc.gpsimd.partition_broadcast(rb[:], d1[:])
                        rc = st.tile([64, SQ], f32, name="rc", tag="rc", bufs=2)
                        nc.vector.reciprocal_approx_fast(rc[:], rb[:])
                        if prow == 0:
                            nc.vector.tensor_mul(
                                ctx_sb[sb][fc][0:64, :], cp[0:64, :], rc[:]
                            )
                        else:
                            nrm = st.tile(
                                [64, SQ], bf16, name="nrm", tag="nrm", bufs=2
                            )
                            nc.vector.tensor_mul(nrm[:], cp[0:64, :], rc[:])
                            # partition shift 0-63 -> 64-127 via SBUF->SBUF DMA
                            nc.sync.dma_start(ctx_sb[sb][fc][64:128, :], nrm[:])

                return [
                    (
                        (lambda b0=b0: score(b0)),
                        (lambda b0=b0: pv(b0)),
                    )
                    for b0 in range(0, nkb, 4)
                ]

            def make_o_thunks(sb, alt_cast=False):
                # alt_cast: alternate the PSUM->bf16 cast between DVE and
                # ScalarE -- in the kernel tail ScalarE is idle and the pp
                # double-buffer rotation is gated on these casts
                thunks = []
                for qb in range(4):
                    for n2 in range(2):

                        def group(qb=qb, n2=n2, sb=sb):
                            pp = psp.tile([128, SQ], f32, name="pp", tag="pp", bufs=2)
                            for fc in range(4):
                                nc.tensor.matmul(
                                    pp[:],
                                    ctx_sb[sb][fc][:, qb * 128 : (qb + 1) * 128],
                                    wo_sb[:, fc * D + n2 * SQ : fc * D + (n2 + 1) * SQ],
                                    start=(fc == 0),
                                    stop=(fc == 3),
                                )
                            ob = st.tile([128, SQ], bf16, name="ob", tag="ob", bufs=2)
                            if alt_cast and (qb * 2 + n2) % 2 == 1:
                                nc.scalar.copy(ob[:], pp[:])
                            else:
                                nc.vector.tensor_copy(ob[:], pp[:])
                            nc.sync.dma_start(
                                out_d[
                                    sb * SQ + qb * 128 : sb * SQ + (qb + 1) * 128,
                                    n2 * SQ : (n2 + 1) * SQ,
                                ],
                                ob[:],
                            )

                        thunks.append(group)
                return thunks

            def clump2(ts):
                # fuse warm thunks in pairs: fewer insertions into the
                # attention stream = fewer accumulation-group boundaries
                # (each boundary exposes an otherwise-hidden LDWEIGHTS)
                out = []
                for i in range(0, len(ts), 2):
                    chunk = ts[i : i + 2]
                    out.append(lambda chunk=chunk: [t() for t in chunk])
                return out

            def pipeline_stream(pairs_by_head, late_proj=None):
                # flatten to S_0, S_1, P_0, S_2, P_1, ... (lag-1 software
                # pipeline). late_proj[g] (this superblock's k/q projection
                # groups 1-3, deferred from the previous superblock) is
                # inserted one full head ahead of the first head that needs
                # it, so its DVE bias-add lands well before those scores.
                stream = []
                prev_p = None
                for hi, hpairs in enumerate(pairs_by_head):
                    if late_proj and hi in (1, 3, 5):
                        stream += late_proj[(hi + 1) // 2]
                    for s_t, p_t in hpairs:
                        stream.append(s_t)
                        if prev_p is not None:
                            stream.append(prev_p)
                        prev_p = p_t
                stream.append(prev_p)
                return stream

            def emit_stream(stream, warm):
                nb, nw = len(stream), len(warm)
                wi = 0
                for bi, bt in enumerate(stream):
                    bt()
                    while wi < nw and (wi + 1) * nb <= (bi + 1) * nw:
                        warm[wi]()
                        wi += 1
                while wi < nw:
                    warm[wi]()
                    wi += 1

            # ---- emission schedule ----
            # sb0: minimal prefix of proj(0) [k0 v0..v3 q0], then attention
            # heads interleaved explicitly with the remaining k/q groups
            # (head pair ti becomes eligible right after k_ti/q_ti), with
            # proj(1) woven across the whole superblock.
            p0 = {
                (pn, g): make_proj_thunk(pn, 0, g)
                for pn in ("k", "v", "q")
                for g in range(4)
            }
            for t in (p0[("k", 0)], p0[("q", 0)]):
                t()
            sp0 = [make_attn_pairs(h, 0)[0] for h in range(HL)]
            s0 = [p[0] for p in sp0]
            v0 = [p[1] for p in sp0]
            # scores for heads 0-1 run while the V projection's x/w still
            # stream in; k1/q1 sit AHEAD of the first PV in the tensor queue
            # so a V wait can never block the next head pair's projections
            stream0 = [
                s0[0], s0[1],
                p0[("v", 0)], p0[("v", 1)], p0[("v", 2)], p0[("v", 3)],
                p0[("k", 1)], p0[("q", 1)],
                s0[2], v0[0], s0[3], v0[1],
                p0[("k", 2)], p0[("q", 2)],
                s0[4], v0[2], s0[5], v0[3],
                p0[("k", 3)], p0[("q", 3)],
                s0[6], v0[4], s0[7], v0[5], v0[6], v0[7],
            ]
            # early proj set E(sc): k/q group 0 + all of v (what the next
            # superblock's first head pair and PVs need). Groups 1-3 of k/q
            # are deferred into the consuming superblock's own stream.
            EARLY = [("k", 0), ("v", 0), ("q", 0), ("v", 1), ("v", 2), ("v", 3)]
            emit_stream(stream0, make_proj_thunks(1, order=EARLY))

            def load_wo():
                nc.sync.dma_start(wo_sb[:], wo_d[:])

            o2 = make_o_thunks(2, alt_cast=True)
            for sb in (1, 2, 3):
                # sb3 ends on even head 6 so the final normalize writes ctx
                # directly (no partition-shift DMA on the critical tail)
                horder = [0, 1, 2, 3, 4, 5, 7, 6] if sb == 3 else range(HL)
                pairs_by_head = [make_attn_pairs(h, sb) for h in horder]
                late = {
                    g: [make_proj_thunk("k", sb, g), make_proj_thunk("q", sb, g)]
                    for g in (1, 2, 3)
                }
                warm = {
                    1: lambda: [load_wo] + make_proj_thunks(2, order=EARLY),
                    2: lambda: make_proj_thunks(3, order=EARLY) + make_o_thunks(0),
                    3: lambda: make_o_thunks(1) + o2[:4],
                }[sb]()
                emit_stream(pipeline_stream(pairs_by_head, late), warm)
            # leftover o(2) groups fill the last normalize's latency
            for t in o2[4:]:
                t()
            for t in make_o_thunks(3, alt_cast=True):
                t()

    nc.compile()
    return nc


def kernel(
    q,
    k,
    v,
    mask=None,
    Wq=None,
    bq=None,
    Wk=None,
    bk=None,
    Wv=None,
    bv=None,
    Wo=None,
    bo=None,
    **_unused,
):
    import ml_dtypes
    from concourse.bass_utils import run_bass_kernel_spmd

    if "nc" not in _cache:
        _cache["nc"] = _build()
    nc = _cache["nc"]

    bf = ml_dtypes.bfloat16
    q = np.asarray(q, np.float32)
    k = np.asarray(k, np.float32)
    v = np.asarray(v, np.float32)
    Wq = np.asarray(Wq, np.float32)
    Wk = np.asarray(Wk, np.float32)
    Wv = np.asarray(Wv, np.float32)
    Wo = np.asarray(Wo, np.float32)
    bq = np.zeros(D, np.float32) if bq is None else np.asarray(bq, np.float32)
    bk = np.zeros(D, np.float32) if bk is None else np.asarray(bk, np.float32)
    bv = np.zeros(D, np.float32) if bv is None else np.asarray(bv, np.float32)
    bo = np.zeros(D, np.float32) if bo is None else np.asarray(bo, np.float32)

    # triangular causal mask [128, 128]: tri[kk, c] = 1 iff kk <= c
    kk = np.arange(128)[:, None]
    cc = np.arange(128)[None, :]
    tri = (kk <= cc).astype(bf)

    def pack_x(x):  # [S, D] -> [NSC, 128, NDM*SQ]  (sc, p, dm*sq)
        xT = np.ascontiguousarray(x.T)  # [D, S]
        return np.ascontiguousarray(
            xT.reshape(NDM, 128, NSC, SQ).transpose(2, 1, 0, 3).reshape(
                NSC, 128, NDM * SQ
            )
        ).astype(bf)

    def pack_w(wT):  # [D, FH] -> [128, NDM*FH]  (p, dm*fh)
        return np.ascontiguousarray(
            wT.reshape(NDM, 128, FH).transpose(1, 0, 2).reshape(128, NDM * FH)
        ).astype(bf)

    xT = {}
    for b in range(B):
        xT[("q", b)] = pack_x(q[b])
        xT[("k", b)] = pack_x(k[b])
        xT[("v", b)] = pack_x(v[b])
    wqs, wks, wvs, wos, bqs = {}, {}, {}, {}, {}
    for hg in range(2):
        sl = slice(hg * FH, (hg + 1) * FH)
        wqs[hg] = pack_w(Wq[sl, :].T * np.float32(0.125))
        wks[hg] = pack_w(Wk[sl, :].T)
        wvs[hg] = pack_w(Wv[sl, :].T)
        # wo: [FH, D] -> [128, 4*D]  (p, fc*d)
        woT = Wo[:, sl].T
        wos[hg] = np.ascontiguousarray(
            woT.reshape(4, 128, D).transpose(1, 0, 2).reshape(128, 4 * D)
        ).astype(bf)
        # packed biases [128, 8]: cols 0-3 = scaled bq chunks, 4-7 = bk
        bb = np.empty((128, 8), np.float32)
        bb[:, 0:4] = (bq[sl] * np.float32(0.125)).reshape(4, 128).T
        bb[:, 4:8] = bk[sl].reshape(4, 128).T
        bqs[hg] = bb
    ones_arr = np.ones((128, SQ), bf)

    in_maps = []
    for c in range(NCORES):
        b, hg = c // 2, c % 2
        in_maps.append(
            {
                "xq": xT[("q", b)],
                "xk": xT[("k", b)],
                "xv": xT[("v", b)],
                "wq": wqs[hg],
                "wk": wks[hg],
                "wv": wvs[hg],
                "wo": wos[hg],
                "tri": tri,
                "bias": bqs[hg],
                "ones": ones_arr,
            }
        )

    res = run_bass_kernel_spmd(nc, in_maps, list(range(NCORES)))
    out = np.empty((B, S, D), np.float32)
    for b in range(B):
        out[b] = np.asarray(res.results[2 * b]["out"], np.float32) + np.asarray(
            res.results[2 * b + 1]["out"], np.float32
        )
    const = Wo @ bv + bo  # bv/bo contribution (folds exactly through softmax)
    if np.any(const):
        out += const[None, None, :]
    return out


# revision 53
# speedup vs baseline: 1.0425x; 1.0064x over previous
"""Multi-head causal attention (B=4, S=2048, D=1024, H=16) on 8 trn2 cores.

Sharding: tensor-parallel over heads x data-parallel over batch.
core c -> (batch b = c//2, head-group hg = c%2 of 8 heads). Every core runs
an identical SPMD program on different data:
  - QKV projections for its 512 features (8 heads). K kept transposed
    [feat, seq] in SBUF; V kept [seq, feat] with a 64-wide ones block per
    head (the PV matmul then emits the softmax denominator for free); Q
    produced per 512-query superblock just in time. Weights live in SBUF
    for the whole kernel (one 1 MB DMA each); x arrives as one prepacked
    DMA per (projection, superblock). Head transfers are split across the
    two hardware DMA queues (ScalarE queue: k/q, which also gate the first
    exps; sync queue: v, ones, and the sb1 x prefetch) since each queue
    executes its transfers serially in emission order.
  - Causal attention per (head, superblock): S^T = K^T.T @ Q^T per 128-key
    block as a single N<=512 bf16 matmul, trimmed to the causally-visible
    query range for diagonal blocks; exp on ScalarE (scores are O(5), exp
    cannot overflow, so no max subtraction) batched over [128,1024] PSUM
    pairs to amortize the 352-cycle ACTIVATE overhead -- the diagonal
    four are ordered (d3,d0,d1,d2) so the first pair's trimmed regions
    are contiguous and share one ACTIVATE; in-place [128,128] triangular
    mask multiply on the partial chunk of diagonal blocks; PV accumulation
    in PSUM (d0 full-width opens the group, trimmed rest). Score and PV
    batches are software-pipelined (PV of batch b is emitted after the
    scores of batch b+1) so the TensorE never waits on the exp stream.
  - ctx^T written straight into persistent SBUF tiles (even heads by DVE
    output placement, odd heads via a small SBUF->SBUF partition-shift DMA)
    -- no DRAM round-trip. Output projection reads those tiles directly,
    and is scheduled late (sb3 + a post-stream remnant) to fill the TensorE
    while the exp-heavy attention tail runs on ScalarE.
Host sums the two partial outputs per batch (the "all-reduce after W_o"
done at gather time) and folds the Wo @ bv + bo constant.

All matmuls run in bf16 (measured 216 ns per N=512 matmul vs 227+ for
f32r, LDWEIGHTS hidden except across PE-geometry switches, no K=64 or
small-N penalties); PSUM accumulation stays fp32, so only input rounding
(~0.4%) is lost, well inside the 2e-2 gate. K=128 matmul density is kept
spread out (no clumping) because dense K=128 bursts trip the HAM 0.5-duty
throttle; K=64 score matmuls don't count toward it.
"""

import sys

import numpy as np

_BASS_PATH = "/opt/trn_rl_repo"
if _BASS_PATH not in sys.path:
    sys.path.insert(0, _BASS_PATH)

B, S, D, H, DK = 4, 2048, 1024, 16, 64
NCORES = 8
FH = 512  # features per core (8 heads)
HL = 8  # local heads
NSC = 4  # seq superblocks of 512
SQ = 512
NKB = 16  # key blocks of 128
NDM = 8  # d_model chunks of 128

_cache = {}


def _build():
    import concourse.bacc as bacc
    import concourse.mybir as mybir
    from concourse.tile import TileContext

    f32, bf16 = mybir.dt.float32, mybir.dt.bfloat16
    AF = mybir.ActivationFunctionType

    nc = bacc.Bacc("TRN2", target_bir_lowering=False, debug=False, num_devices=1)

    # x prepacked as [sc, p, dm*SQ]; w prepacked as [p, dm*FH]
    xq_d = nc.dram_tensor("xq", [NSC, 128, NDM * SQ], bf16, kind="ExternalInput").ap()
    xk_d = nc.dram_tensor("xk", [NSC, 128, NDM * SQ], bf16, kind="ExternalInput").ap()
    xv_d = nc.dram_tensor("xv", [NSC, 128, NDM * SQ], bf16, kind="ExternalInput").ap()
    wq_d = nc.dram_tensor("wq", [128, NDM * FH], bf16, kind="ExternalInput").ap()
    wk_d = nc.dram_tensor("wk", [128, NDM * FH], bf16, kind="ExternalInput").ap()
    wv_d = nc.dram_tensor("wv", [128, NDM * FH], bf16, kind="ExternalInput").ap()
    wo_d = nc.dram_tensor("wo", [128, 4 * D], bf16, kind="ExternalInput").ap()
    # triangular causal mask [128, 128]: tri[k, c] = 1 iff k <= c
    tri_d = nc.dram_tensor("tri", [128, 128], bf16, kind="ExternalInput").ap()
    # packed biases [128, 8]: cols 0-3 = bq chunks, cols 4-7 = bk chunks
    bias_d = nc.dram_tensor("bias", [128, 8], f32, kind="ExternalInput").ap()
    ones_d = nc.dram_tensor("ones", [128, SQ], bf16, kind="ExternalInput").ap()
    out_d = nc.dram_tensor("out", [S, D], bf16, kind="ExternalOutput").ap()

    with TileContext(nc) as tc:
        with (
            tc.tile_pool(name="res", bufs=1) as res,
            tc.tile_pool(name="st", bufs=1) as st,
            tc.tile_pool(name="psum", bufs=1, space="PSUM") as psp,
        ):
            # ---- persistent tiles + head DMAs (ordered for earliest start)
            # tiny consts first so they don't queue behind the 6 MB of x/w
            tri = res.tile([128, 128], bf16, name="tri", tag="tri")
            nc.sync.dma_start(tri[:], tri_d[:])
            bias_sb = res.tile([128, 8], f32, name="bias", tag="bias")
            nc.sync.dma_start(bias_sb[:], bias_d[:])
            bq_t = [bias_sb[:, i : i + 1] for i in range(4)]
            bk_t = [bias_sb[:, 4 + i : 5 + i] for i in range(4)]

            w_sb = {}
            xbox = {}
            HX = NDM * SQ // 2

            def emit_x_load(pname, x_d, sc, halves=False):
                xt = st.tile(
                    [128, NDM * SQ], bf16, name=f"x{pname}", tag=f"x{pname}", bufs=2
                )
                if halves:
                    nc.sync.dma_start(xt[:, 0:HX], x_d[sc][:, 0:HX])
                    nc.sync.dma_start(xt[:, HX:], x_d[sc][:, HX:])
                else:
                    nc.sync.dma_start(xt[:], x_d[sc])
                xbox[(pname, sc)] = xt

            # Two independent DMA queues (sync + scalar). Head plan:
            #   scalar queue: k and q w/x (gates the first scores AND the
            #     first exps, so borrowing ScalarE costs nothing)
            #   sync queue: v w/x, ones blocks, then sc=1's x prefetches
            # so sb1 never stalls behind the head transfers.
            for pname, w_d, x_d in (("k", wk_d, xk_d), ("q", wq_d, xq_d)):
                wt = res.tile([128, NDM * FH], bf16, name=f"w{pname}", tag=f"w{pname}")
                nc.scalar.dma_start(wt[:, 0:HX], w_d[:, 0:HX])
                w_sb[pname] = wt
                xt = st.tile(
                    [128, NDM * SQ], bf16, name=f"x{pname}", tag=f"x{pname}", bufs=2
                )
                nc.scalar.dma_start(xt[:, 0:HX], x_d[0][:, 0:HX])
                xbox[(pname, 0)] = xt
                nc.scalar.dma_start(wt[:, HX:], w_d[:, HX:])
                nc.scalar.dma_start(xt[:, HX:], x_d[0][:, HX:])
            wt = res.tile([128, NDM * FH], bf16, name="wv", tag="wv")
            nc.sync.dma_start(wt[:, 0:HX], wv_d[:, 0:HX])
            w_sb["v"] = wt
            xt = st.tile([128, NDM * SQ], bf16, name="xv", tag="xv", bufs=2)
            nc.sync.dma_start(xt[:, 0:HX], xv_d[0][:, 0:HX])
            xbox[("v", 0)] = xt
            nc.sync.dma_start(wt[:, HX:], wv_d[:, HX:])
            nc.sync.dma_start(xt[:, HX:], xv_d[0][:, HX:])
            # wo is not needed until o(0) in sb2; loaded with sb1's warm
            wo_sb = res.tile([128, 4 * D], bf16, name="wo", tag="wo")

            kt = [res.tile([128, S], bf16, name=f"kt{i}", tag=f"kt{i}") for i in range(4)]
            # per head: 64 V columns + 64 ones columns. The PV matmul then
            # yields the softmax denominator replicated on partitions 64-127
            # (no partition_broadcast needed for the normalize).
            vaug = [
                res.tile([128, HL * 128], bf16, name=f"va{k}", tag=f"va{k}")
                for k in range(NKB)
            ]
            # persistent ctx^T tiles: ctx_sb[sb][fc] is [128, 512]
            ctx_sb = [
                [
                    res.tile([128, SQ], bf16, name=f"cx{sb}{fc}", tag=f"cx{sb}{fc}")
                    for fc in range(4)
                ]
                for sb in range(NSC)
            ]
            # ones blocks land via DMA (no compute-engine queue time). The
            # head region is chip-HBM-bound, so order strictly by need time:
            # kb0-7 (sb0 + sb1-start PVs), then sb1's x prefetch, then the
            # rest of the ones blocks.
            ones_src = ones_d.rearrange("p (h o) -> p h o", o=64)

            def ones_dma(kb):
                va3 = vaug[kb][:, :].rearrange("p (h e) -> p h e", e=128)
                nc.sync.dma_start(va3[:, :, 64:128], ones_src)

            for kb in range(8):
                ones_dma(kb)
            emit_x_load("k", xk_d, 1)
            emit_x_load("v", xv_d, 1)
            emit_x_load("q", xq_d, 1)
            for kb in range(8, NKB):
                ones_dma(kb)

            qsf_box = {}  # sc -> [4 q-slice tiles]

            def make_proj_thunk(pname, sc, gi):
                def group(pname=pname, sc=sc, gi=gi):
                    if pname == "q" and gi == 0:
                        qsf_box[sc] = [
                            st.tile(
                                [128, SQ], bf16, name=f"qs{i}", tag=f"qs{i}", bufs=2
                            )
                            for i in range(4)
                        ]
                    xt = xbox[(pname, sc)]
                    w = w_sb[pname]
                    if pname in ("q", "k"):
                        pp = psp.tile([128, SQ], f32, name="pp", tag="pp", bufs=2)
                        for dm in range(NDM):
                            c0 = dm * FH + gi * 128
                            nc.tensor.matmul(
                                pp[:],
                                w[:, c0 : c0 + 128],
                                xt[:, dm * SQ : (dm + 1) * SQ],
                                start=(dm == 0),
                                stop=(dm == NDM - 1),
                            )
                        if pname == "k":
                            nc.vector.tensor_scalar_add(
                                kt[gi][:, sc * SQ : (sc + 1) * SQ], pp[:], bk_t[gi]
                            )
                        else:
                            nc.vector.tensor_scalar_add(
                                qsf_box[sc][gi][:], pp[:], bq_t[gi]
                            )
                    else:  # v
                        kb = sc * 4 + gi
                        pp = psp.tile([128, FH], f32, name="pp", tag="pp", bufs=2)
                        for dm in range(NDM):
                            c0 = dm * SQ + gi * 128
                            nc.tensor.matmul(
                                pp[:],
                                xt[:, c0 : c0 + 128],
                                w[:, dm * FH : (dm + 1) * FH],
                                start=(dm == 0),
                                stop=(dm == NDM - 1),
                            )
                        va3 = vaug[kb][:, :].rearrange("p (h e) -> p h e", e=128)
                        pp3 = pp[:].rearrange("p (h e) -> p h e", e=64)
                        nc.vector.tensor_copy(va3[:, :, 0:64], pp3[:])

                return group

            def make_proj_thunks(sc, order=None):
                if order is None:
                    # round-robin k/v/q so the q bias-adds (which gate the
                    # next superblock's first scores) land early in the DVE
                    # queue, and vaug tiles spread across the superblock
                    order = [
                        (pn, g) for g in range(4) for pn in ("k", "v", "q")
                    ]
                thunks = []
                loads_done = set()
                for pname, gi in order:
                    if pname not in loads_done and sc > 1:
                        # x load for this (pname, sc) rides with its first group
                        loads_done.add(pname)
                        xd = {"k": xk_d, "v": xv_d, "q": xq_d}[pname]

                        def lg(pname=pname, xd=xd, sc=sc, gi=gi):
                            emit_x_load(pname, xd, sc)
                            make_proj_thunk(pname, sc, gi)()

                        thunks.append(lg)
                    else:
                        thunks.append(make_proj_thunk(pname, sc, gi))
                return thunks

            def make_attn_pairs(h, sb):
                """(score_thunk, pv_thunk) pairs for one (head, superblock).

                kb order: diagonal blocks first [d0..d3] (d0 full query width
                and start=True; d1..d3 trimmed to the causally visible query
                range), then off-diagonal blocks.
                """
                ti, po = h // 2, (h % 2) * 64
                nkb = 4 * (sb + 1)
                # diagonal order (d3, d0, d1, d2): the first pair's trimmed
                # exp regions [384:512] and [512:1024] merge into ONE
                # contiguous ACTIVATE; PV still consumes d0 first (start=True
                # needs d0's full query width)
                b = 4 * sb
                kbs = [b + 3, b + 0, b + 1, b + 2] + list(range(4 * sb))
                state = {}

                def score(b0):
                    esl = []
                    for pi in range(2):  # two kb-pairs per batch
                        sp = psp.tile(
                            [128, 2 * SQ], f32, name="sp", tag="sp", bufs=2
                        )
                        es = st.tile(
                            [128, 2 * SQ], bf16, name="es", tag="es", bufs=8
                        )
                        q0s = []
                        for j in range(2):
                            kb = kbs[b0 + 2 * pi + j]
                            dj = kb - sb * 4  # diagonal index (0..3) or neg
                            q0 = dj * 128 if 0 <= dj else 0
                            nc.tensor.matmul(
                                sp[:, j * SQ + q0 : (j + 1) * SQ],
                                kt[ti][po : po + 64, kb * 128 : (kb + 1) * 128],
                                qsf_box[sb][ti][po : po + 64, q0:SQ],
                                start=True,
                                stop=True,
                            )
                            q0s.append((kb, dj, q0))
                        if all(dj < 0 for _, dj, _ in q0s):
                            # off-diagonal pair: one wide exp (352-cycle
                            # ACTIVATE overhead amortized over 1024 cols)
                            nc.scalar.activation(es[:], sp[:], AF.Exp)
                        elif q0s[0][1] == 3 and q0s[1][1] == 0:
                            # (d3, d0) pair: valid regions [384:512]+[512:1024]
                            # are contiguous -> one exp
                            nc.scalar.activation(
                                es[:, 384:1024], sp[:, 384:1024], AF.Exp
                            )
                        else:
                            for j, (kb, dj, q0) in enumerate(q0s):
                                nc.scalar.activation(
                                    es[:, j * SQ + q0 : (j + 1) * SQ],
                                    sp[:, j * SQ + q0 : (j + 1) * SQ],
                                    AF.Exp,
                                )
                        for j, (kb, dj, q0) in enumerate(q0s):
                            if 0 <= dj:
                                c = j * SQ + q0
                                nc.vector.tensor_mul(
                                    es[:, c : c + 128], es[:, c : c + 128], tri[:]
                                )
                            esl.append((kb, q0, es, j))
                    state[b0] = esl

                def pv(b0):
                    esl = state.pop(b0)
                    if b0 == 0:
                        state["cp"] = psp.tile(
                            [128, SQ], f32, name="cp", tag="cp", bufs=2
                        )
                        state["emitted"] = 0
                        # d0 (full query width) must open the accumulation
                        esl = [esl[1], esl[0], esl[2], esl[3]]
                    cp = state["cp"]
                    for kb, q0, es, j in esl:
                        nc.tensor.matmul(
                            cp[:, q0:SQ],
                            vaug[kb][:, h * 128 : (h + 1) * 128],
                            es[:, j * SQ + q0 : (j + 1) * SQ],
                            start=(state["emitted"] == 0),
                            stop=(state["emitted"] == nkb - 1),
                        )
                        state["emitted"] += 1
                    if b0 + 4 >= nkb:
                        # normalize and write ctx^T slice into SBUF ctx tiles
                        fc, prow = h // 2, (h % 2) * 64
                        d1 = st.tile([1, SQ], f32, name="d1", tag="d1", bufs=2)
                        nc.vector.tensor_copy(d1[:], cp[64:65, :])
                        rb = st.tile([64, SQ], f32, name="rb", tag="rb", bufs=2)
                        nc.gpsimd.partition_broadcast(rb[:], d1[:])
                        rc = st.tile([64, SQ], f32, name="rc", tag="rc", bufs=2)
                        nc.vector.reciprocal_approx_fast(rc[:], rb[:])
                        if prow == 0:
                            nc.vector.tensor_mul(
                                ctx_sb[sb][fc][0:64, :], cp[0:64, :], rc[:]
                            )
                        else:
                            nrm = st.tile(
                                [64, SQ], bf16, name="nrm", tag="nrm", bufs=2
                            )
                            nc.vector.tensor_mul(nrm[:], cp[0:64, :], rc[:])
                            # partition shift 0-63 -> 64-127 via SBUF->SBUF DMA
                            nc.sync.dma_start(ctx_sb[sb][fc][64:128, :], nrm[:])

                return [
                    (
                        (lambda b0=b0: score(b0)),
                        (lambda b0=b0: pv(b0)),
                    )
                    for b0 in range(0, nkb, 4)
                ]

            def make_o_thunks(sb, alt_cast=False):
                # alt_cast: alternate the PSUM->bf16 cast between DVE and
                # ScalarE -- in the kernel tail ScalarE is idle and the pp
                # double-buffer rotation is gated on these casts
                thunks = []
                for qb in range(4):
                    for n2 in range(2):

                        def group(qb=qb, n2=n2, sb=sb):
                            pp = psp.tile([128, SQ], f32, name="pp", tag="pp", bufs=2)
                            for fc in range(4):
                                nc.tensor.matmul(
                                    pp[:],
                                    ctx_sb[sb][fc][:, qb * 128 : (qb + 1) * 128],
                                    wo_sb[:, fc * D + n2 * SQ : fc * D + (n2 + 1) * SQ],
                                    start=(fc == 0),
                                    stop=(fc == 3),
                                )
                            ob = st.tile([128, SQ], bf16, name="ob", tag="ob", bufs=2)
                            if alt_cast and (qb * 2 + n2) % 2 == 1:
                                nc.scalar.copy(ob[:], pp[:])
                            else:
                                nc.vector.tensor_copy(ob[:], pp[:])
                            nc.sync.dma_start(
                                out_d[
                                    sb * SQ + qb * 128 : sb * SQ + (qb + 1) * 128,
                                    n2 * SQ : (n2 + 1) * SQ,
                                ],
                                ob[:],
                            )

                        thunks.append(group)
                return thunks

            def clump2(ts):
                # fuse warm thunks in pairs: fewer insertions into the
                # attention stream = fewer accumulation-group boundaries
                # (each boundary exposes an otherwise-hidden LDWEIGHTS)
                out = []
                for i in range(0, len(ts), 2):
                    chunk = ts[i : i + 2]
                    out.append(lambda chunk=chunk: [t() for t in chunk])
                return out

            def pipeline_stream(pairs_by_head, late_proj=None):
                # flatten to S_0, S_1, P_0, S_2, P_1, ... (lag-1 software
                # pipeline). late_proj[g] (this superblock's k/q projection
                # groups 1-3, deferred from the previous superblock) is
                # inserted one full head ahead of the first head that needs
                # it, so its DVE bias-add lands well before those scores.
                stream = []
                prev_p = None
                for hi, hpairs in enumerate(pairs_by_head):
                    if late_proj and hi in (1, 3, 5):
                        stream += late_proj[(hi + 1) // 2]
                    for s_t, p_t in hpairs:
                        stream.append(s_t)
                        if prev_p is not None:
                            stream.append(prev_p)
                        prev_p = p_t
                stream.append(prev_p)
                return stream

            def emit_stream(stream, warm):
                nb, nw = len(stream), len(warm)
                wi = 0
                for bi, bt in enumerate(stream):
                    bt()
                    while wi < nw and (wi + 1) * nb <= (bi + 1) * nw:
                        warm[wi]()
                        wi += 1
                while wi < nw:
                    warm[wi]()
                    wi += 1

            # ---- emission schedule ----
            # sb0: minimal prefix of proj(0) [k0 v0..v3 q0], then attention
            # heads interleaved explicitly with the remaining k/q groups
            # (head pair ti becomes eligible right after k_ti/q_ti), with
            # proj(1) woven across the whole superblock.
            p0 = {
                (pn, g): make_proj_thunk(pn, 0, g)
                for pn in ("k", "v", "q")
                for g in range(4)
            }
            for t in (p0[("k", 0)], p0[("q", 0)]):
                t()
            sp0 = [make_attn_pairs(h, 0)[0] for h in range(HL)]
            s0 = [p[0] for p in sp0]
            v0 = [p[1] for p in sp0]
            # scores for heads 0-1 run while the V projection's x/w still
            # stream in; k1/q1 sit AHEAD of the first PV in the tensor queue
            # so a V wait can never block the next head pair's projections
            # k groups 1-3 need only the k data (landed first on the scalar
            # DMA queue) -- run them in the otherwise-idle window while q/v
            # still stream in; q groups slot just-in-time before the head
            # pairs that read them, V projections before the first PV
            stream0 = [
                p0[("k", 1)], s0[0], s0[1],
                p0[("k", 2)], p0[("q", 1)],
                p0[("v", 0)], p0[("v", 1)], p0[("v", 2)], p0[("v", 3)],
                v0[0],
                p0[("k", 3)], p0[("q", 2)],
                s0[2], v0[1], s0[3], v0[2],
                p0[("q", 3)],
                s0[4], v0[3], s0[5], v0[4],
                s0[6], v0[5], s0[7], v0[6], v0[7],
            ]
            # early proj set E(sc): k/q group 0 + all of v (what the next
            # superblock's first head pair and PVs need). Groups 1-3 of k/q
            # are deferred into the consuming superblock's own stream.
            EARLY = [("k", 0), ("v", 0), ("q", 0), ("v", 1), ("v", 2), ("v", 3)]
            emit_stream(stream0, make_proj_thunks(1, order=EARLY))

            def load_wo():
                nc.sync.dma_start(wo_sb[:], wo_d[:])

            o2 = make_o_thunks(2, alt_cast=True)
            for sb in (1, 2, 3):
                # sb3 ends on even head 6 so the final normalize writes ctx
                # directly (no partition-shift DMA on the critical tail)
                horder = [0, 1, 2, 3, 4, 5, 7, 6] if sb == 3 else range(HL)
                pairs_by_head = [make_attn_pairs(h, sb) for h in horder]
                late = {
                    g: [make_proj_thunk("k", sb, g), make_proj_thunk("q", sb, g)]
                    for g in (1, 2, 3)
                }
                warm = {
                    1: lambda: [load_wo] + make_proj_thunks(2, order=EARLY),
                    2: lambda: make_proj_thunks(3, order=EARLY) + make_o_thunks(0),
                    3: lambda: make_o_thunks(1) + o2[:4],
                }[sb]()
                emit_stream(pipeline_stream(pairs_by_head, late), warm)
            # leftover o(2) groups fill the last normalize's latency
            for t in o2[4:]:
                t()
            for t in make_o_thunks(3, alt_cast=True):
                t()

    nc.compile()
    return nc


def kernel(
    q,
    k,
    v,
    mask=None,
    Wq=None,
    bq=None,
    Wk=None,
    bk=None,
    Wv=None,
    bv=None,
    Wo=None,
    bo=None,
    **_unused,
):
    import ml_dtypes
    from concourse.bass_utils import run_bass_kernel_spmd

    if "nc" not in _cache:
        _cache["nc"] = _build()
    nc = _cache["nc"]

    bf = ml_dtypes.bfloat16
    q = np.asarray(q, np.float32)
    k = np.asarray(k, np.float32)
    v = np.asarray(v, np.float32)
    Wq = np.asarray(Wq, np.float32)
    Wk = np.asarray(Wk, np.float32)
    Wv = np.asarray(Wv, np.float32)
    Wo = np.asarray(Wo, np.float32)
    bq = np.zeros(D, np.float32) if bq is None else np.asarray(bq, np.float32)
    bk = np.zeros(D, np.float32) if bk is None else np.asarray(bk, np.float32)
    bv = np.zeros(D, np.float32) if bv is None else np.asarray(bv, np.float32)
    bo = np.zeros(D, np.float32) if bo is None else np.asarray(bo, np.float32)

    # triangular causal mask [128, 128]: tri[kk, c] = 1 iff kk <= c
    kk = np.arange(128)[:, None]
    cc = np.arange(128)[None, :]
    tri = (kk <= cc).astype(bf)

    def pack_x(x):  # [S, D] -> [NSC, 128, NDM*SQ]  (sc, p, dm*sq)
        xT = np.ascontiguousarray(x.T)  # [D, S]
        return np.ascontiguousarray(
            xT.reshape(NDM, 128, NSC, SQ).transpose(2, 1, 0, 3).reshape(
                NSC, 128, NDM * SQ
            )
        ).astype(bf)

    def pack_w(wT):  # [D, FH] -> [128, NDM*FH]  (p, dm*fh)
        return np.ascontiguousarray(
            wT.reshape(NDM, 128, FH).transpose(1, 0, 2).reshape(128, NDM * FH)
        ).astype(bf)

    xT = {}
    for b in range(B):
        xT[("q", b)] = pack_x(q[b])
        xT[("k", b)] = pack_x(k[b])
        xT[("v", b)] = pack_x(v[b])
    wqs, wks, wvs, wos, bqs = {}, {}, {}, {}, {}
    for hg in range(2):
        sl = slice(hg * FH, (hg + 1) * FH)
        wqs[hg] = pack_w(Wq[sl, :].T * np.float32(0.125))
        wks[hg] = pack_w(Wk[sl, :].T)
        wvs[hg] = pack_w(Wv[sl, :].T)
        # wo: [FH, D] -> [128, 4*D]  (p, fc*d)
        woT = Wo[:, sl].T
        wos[hg] = np.ascontiguousarray(
            woT.reshape(4, 128, D).transpose(1, 0, 2).reshape(128, 4 * D)
        ).astype(bf)
        # packed biases [128, 8]: cols 0-3 = scaled bq chunks, 4-7 = bk
        bb = np.empty((128, 8), np.float32)
        bb[:, 0:4] = (bq[sl] * np.float32(0.125)).reshape(4, 128).T
        bb[:, 4:8] = bk[sl].reshape(4, 128).T
        bqs[hg] = bb
    ones_arr = np.ones((128, SQ), bf)

    in_maps = []
    for c in range(NCORES):
        b, hg = c // 2, c % 2
        in_maps.append(
            {
                "xq": xT[("q", b)],
                "xk": xT[("k", b)],
                "xv": xT[("v", b)],
                "wq": wqs[hg],
                "wk": wks[hg],
                "wv": wvs[hg],
                "wo": wos[hg],
                "tri": tri,
                "bias": bqs[hg],
                "ones": ones_arr,
            }
        )

    res = run_bass_kernel_spmd(nc, in_maps, list(range(NCORES)))
    out = np.empty((B, S, D), np.float32)
    for b in range(B):
        out[b] = np.asarray(res.results[2 * b]["out"], np.float32) + np.asarray(
            res.results[2 * b + 1]["out"], np.float32
        )
    const = Wo @ bv + bo  # bv/bo contribution (folds exactly through softmax)
    if np.any(const):
        out += const[None, None, :]
    return out
